# revision 1
# baseline (speedup 1.0000x reference)
"""Trainium2 Bass kernel for a dense transformer block (B=4, N=1024, D=1024,
H=16, Dh=64, MLP 4x), distributed over 8 NeuronCores with ZERO collectives.

Sharding: core c handles batch b = c//2, sequence half = c%2 (512 query
rows).  K/V are computed for the batch's full 1024-token sequence on both
cores of a pair (the ~12% duplicated K/V FLOPs are far cheaper than the
~190us/16MB AllReduce the tensor-parallel split would need twice).  The
sequence is rotated per-core so the core's own 512 rows are always rows
0..511 of its input — attention is permutation-invariant over keys, so all
8 cores run one identical SPMD program.

Compute layout: residual stream stays natural [seq, d] in f32.  LN outputs
enter the transposed domain ([d, seq] bf16) via DMA-transpose bounced
through DRAM; Q^T/K^T/V and the MLP hidden G^T are produced transposed, and
the output projections (Wo, Wproj) consume the transposed activations as
the matmul's stationary operand, producing NATURAL-layout outputs whose
PSUM->SBUF copy is fused with the residual add.  Matmuls run in bf16 (PSUM
f32); softmax skips max-subtraction (scores ~N(0,0.4^2)) and normalizes
attention output after the AV matmul using a ones-column appended to V for
the denominators.
"""

import numpy as np

import bass_rust
import concourse.bass as bass
import concourse.mybir as mybir
import concourse.tile as tile
from concourse.masks import make_identity

F32 = mybir.dt.float32
BF16 = mybir.dt.bfloat16
AF = mybir.ActivationFunctionType
ALU = mybir.AluOpType

P = 128
D = 1024
S = 1024          # full sequence (per batch)
SO = 512          # own rows per core
H = 16
DH = 64
F = 4096
EPS = 1e-5
N_CORES = 8

ND = D // P       # 8   d tiles
NS = S // P       # 8   full-seq tiles
NSO = SO // P     # 4   own-seq tiles
NF = F // P       # 32  ff tiles


# --------------------------------------------------------------------------
# Workaround: this compiler build supports only ONE semaphore wait per
# instruction.  Move excess waits onto fresh NOPs inserted just before the
# offending instruction on the same engine.
# --------------------------------------------------------------------------
_counter = [0]


def _split_multiwaits(nc):
    nsplit = 0
    for fn in nc.m.functions:
        for blk in fn.blocks:
            il = list(blk.instructions)
            out = []
            changed = False
            for inst in il:
                si = inst.sync_info
                if si is not None and len(si.on_wait) > 1:
                    waits = list(si.on_wait)
                    for w in waits[:-1]:
                        _counter[0] += 1
                        nop = mybir.InstNoOp(
                            name=f"I-waitsplit-{_counter[0]}", ins=[], outs=[]
                        )
                        nop.engine = inst.engine
                        nop.sync_info = bass_rust.SyncInfo(on_wait=[w], on_update=[])
                        out.append(nop)
                        nc.register_instruction(nop, overwrite=True)
                    inst.sync_info = bass_rust.SyncInfo(
                        on_wait=[waits[-1]], on_update=list(si.on_update)
                    )
                    changed = True
                    nsplit += 1
                out.append(inst)
            if changed:
                blk.instructions = out
    return nsplit


def _vec_tile(nc, pool, ext, n):
    """Load a [n*128] dram vector as a [128, n] sbuf tile (col i = tile i)."""
    t = pool.tile([P, n], F32, name=ext.name + "_sb")
    nc.sync.dma_start(out=t[:], in_=ext[:].rearrange("(o p) -> p o", p=P))
    return t


def _bcast_tile(nc, pool, ext, n):
    """Load a [n] dram vector broadcast to a [128, n] sbuf tile."""
    t = pool.tile([P, n], F32, name=ext.name + "_bc")
    ap = ext[:]
    src = bass.AP(tensor=ap.tensor, offset=ap.offset, ap=[[0, P], ap.ap[0]])
    nc.sync.dma_start(out=t[:], in_=src)
    return t


def build():
    nc = bass.Bass(name="tfblock")

    x_ext = nc.declare_dram_parameter("x", [S, D], F32, isOutput=False)
    ln1_w = nc.declare_dram_parameter("ln1_w", [D], F32, isOutput=False)
    ln1_b = nc.declare_dram_parameter("ln1_b", [D], F32, isOutput=False)
    Wq_e = nc.declare_dram_parameter("Wq", [D, D], F32, isOutput=False)
    bq_e = nc.declare_dram_parameter("bq", [D], F32, isOutput=False)
    Wk_e = nc.declare_dram_parameter("Wk", [D, D], F32, isOutput=False)
    bk_e = nc.declare_dram_parameter("bk", [D], F32, isOutput=False)
    Wv_e = nc.declare_dram_parameter("Wv", [D, D], F32, isOutput=False)
    bv_e = nc.declare_dram_parameter("bv", [D], F32, isOutput=False)
    Wo_e = nc.declare_dram_parameter("Wo", [D, D], F32, isOutput=False)
    bo_e = nc.declare_dram_parameter("bo", [D], F32, isOutput=False)
    ln2_w = nc.declare_dram_parameter("ln2_w", [D], F32, isOutput=False)
    ln2_b = nc.declare_dram_parameter("ln2_b", [D], F32, isOutput=False)
    Wfc_e = nc.declare_dram_parameter("Wfc", [D, F], F32, isOutput=False)
    bfc_e = nc.declare_dram_parameter("bfc", [F], F32, isOutput=False)
    Wp_e = nc.declare_dram_parameter("Wproj", [F, D], F32, isOutput=False)
    bp_e = nc.declare_dram_parameter("bproj", [D], F32, isOutput=False)
    out_ext = nc.declare_dram_parameter("out", [SO, D], F32, isOutput=True)

    cast_cycle = [0]

    def copy_cast(out, in_, eng=None):
        if eng is None:
            eng = ("v", "g", "s")[cast_cycle[0] % 3]
            cast_cycle[0] += 1
        e = {"v": 0, "g": 1, "s": 2}[eng]
        if e == 0:
            nc.vector.tensor_copy(out=out, in_=in_)
        elif e == 1:
            nc.gpsimd.tensor_copy(out=out, in_=in_)
        else:
            nc.scalar.copy(out=out, in_=in_)

    def ln_tile(lnp, src_ap, hn_out, eps_t, tag):
        """LayerNorm stats on DVE + apply on ACT: hn_out = (src-mu)*rstd."""
        stats = lnp.tile([P, 2, 6], F32, tag=tag + "_st")
        for g in range(2):
            nc.vector.bn_stats(out=stats[:, g, :], in_=src_ap[:, g * 512 : (g + 1) * 512])
        mv = lnp.tile([P, 2], F32, tag=tag + "_mv")
        nc.vector.bn_aggr(out=mv[:], in_=stats[:])
        lnv = lnp.tile([P, 1], F32, tag=tag + "_sd")
        nc.scalar.activation(out=lnv[:], in_=mv[:, 1:2], func=AF.Ln, bias=eps_t[:])
        rstd = lnp.tile([P, 1], F32, tag=tag + "_rs")
        nc.scalar.activation(out=rstd[:], in_=lnv[:], func=AF.Exp, scale=-0.5)
        nb = lnp.tile([P, 1], F32, tag=tag + "_nb")
        nc.vector.tensor_scalar(nb[:], mv[:, 0:1], rstd[:], -1.0, ALU.mult, ALU.mult)
        nc.scalar.activation(
            out=hn_out, in_=src_ap, func=AF.Identity, bias=nb[:], scale=rstd[:]
        )

    with tile.TileContext(nc) as tc:
        from contextlib import ExitStack

        with ExitStack() as top:
            consts = top.enter_context(tc.tile_pool(name="consts", bufs=1))
            persist = top.enter_context(tc.tile_pool(name="persist", bufs=1))
            dram = top.enter_context(tc.tile_pool(name="dram", bufs=1, space="DRAM"))

            ln1w_t = _vec_tile(nc, consts, ln1_w, ND)
            ln1b_t = _vec_tile(nc, consts, ln1_b, ND)
            ln2w_t = _vec_tile(nc, consts, ln2_w, ND)
            ln2b_t = _vec_tile(nc, consts, ln2_b, ND)
            bq_t = _vec_tile(nc, consts, bq_e, ND)
            bk_t = _vec_tile(nc, consts, bk_e, ND)
            bfc_t = _vec_tile(nc, consts, bfc_e, NF)
            bv_bc = _bcast_tile(nc, consts, bv_e, D)

            eps_t = consts.tile([P, 1], F32, name="eps")
            nc.vector.memset(eps_t[:], EPS)
            e0 = consts.tile([P, P], F32, name="e0")
            nc.vector.memset(e0[:], 0.0)
            nc.vector.memset(e0[0:1, :], 1.0)
            ident = consts.tile([P, P], BF16, name="ident")
            make_identity(nc, ident[:])

            # xN_own lives until residual 1 (pre-biased with bo);
            # QT/KT/VN live until end of the Wo projection.
            xown_cm = tc.tile_pool(name="xown", bufs=1)
            xown = xown_cm.__enter__()
            xN_own = xown.tile([P, NSO, D], F32, name="xN_own")
            nc.sync.dma_start(
                out=xN_own[:], in_=x_ext[0:SO, :].rearrange("(t p) d -> p t d", p=P)
            )
            x1N = persist.tile([P, NSO, D], F32, name="x1N")

            qkv_cm = tc.tile_pool(name="qkvp", bufs=1)
            qkvp = qkv_cm.__enter__()

            # ------------------------- LN1 (keeps hn in SBUF), weights, QKV
            with ExitStack() as phB:
                wpool = phB.enter_context(tc.tile_pool(name="wqkv", bufs=1))
                stg = phB.enter_context(tc.tile_pool(name="stgB", bufs=2))
                psB = phB.enter_context(tc.tile_pool(name="psumB", bufs=2, space="PSUM"))
                hTp = phB.enter_context(tc.tile_pool(name="hTp", bufs=1))

                # LN1 per tile, with the h^T PE-transposes (ln1 w/b fused in
                # the DVE copy-back) interleaved so PE warms up immediately
                hnN = hTp.tile([P, NS, D], BF16, name="hnN")
                hT_own = hTp.tile([P, ND, SO], BF16, name="hT_own")
                hT_oth = hTp.tile([P, ND, SO], BF16, name="hT_oth")
                with tc.tile_pool(name="ln1", bufs=2) as lnp:
                    for st in range(NS):
                        xt = lnp.tile([P, D], F32, tag="xt")
                        nc.sync.dma_start(out=xt[:], in_=x_ext[st * P : (st + 1) * P, :])
                        ln_tile(lnp, xt[:], hnN[:, st, :], eps_t, "l1")
                        hTx = hT_own if st < 4 else hT_oth
                        st4 = st % 4
                        for dt in range(ND):
                            pst = psB.tile([P, P], BF16, tag="ps_t")
                            nc.tensor.transpose(
                                pst[:], hnN[:, st, dt * P : (dt + 1) * P], ident[:]
                            )
                            nc.vector.tensor_scalar(
                                hTx[:, dt, st4 * P : (st4 + 1) * P],
                                pst[:],
                                ln1w_t[:, dt : dt + 1],
                                ln1b_t[:, dt : dt + 1],
                                ALU.mult,
                                ALU.add,
                            )

                Wq_bf = wpool.tile([P, ND, D], BF16, name="Wq_bf")
                Wk_bf = wpool.tile([P, ND, D], BF16, name="Wk_bf")
                Wv_bf = wpool.tile([P, ND, D], BF16, name="Wv_bf")
                for w_ext, w_bf, engs in (
                    (Wq_e, Wq_bf, ("v", "s")),
                    (Wk_e, Wk_bf, ("g",)),
                    (Wv_e, Wv_bf, ("v", "s")),
                ):
                    for kt in range(ND):
                        s = stg.tile([P, D], F32, tag="wstg")
                        nc.sync.dma_start(out=s[:], in_=w_ext[kt * P : (kt + 1) * P, :])
                        copy_cast(w_bf[:, kt, :], s[:], eng=engs[kt % len(engs)])

                QT = qkvp.tile([P, ND, SO], BF16, name="QT")
                KTe = qkvp.tile([P, ND, S], BF16, name="KTe")
                KTo = qkvp.tile([P, ND, S], BF16, name="KTo")
                VN = qkvp.tile([P, NS, H, P], BF16, name="VN")
                nc.gpsimd.memset(KTe[64:128, :, :], 0.0)
                nc.gpsimd.memset(KTo[0:64, :, :], 0.0)
                nc.vector.memset(VN[:, :, :, DH + 1 :], 0.0)
                nc.vector.memset(VN[:, :, :, DH : DH + 1], 1.0)

                for ot in range(ND):
                    ps = psB.tile([P, 512], F32, tag="ps_q")
                    for kt in range(ND):
                        nc.tensor.matmul(
                            ps[:],
                            Wq_bf[:, kt, ot * P : (ot + 1) * P],
                            hT_own[:, kt, :],
                            start=(kt == 0),
                            stop=(kt == ND - 1),
                        )
                    nc.vector.tensor_scalar(
                        QT[:, ot, :], ps[:], bq_t[:, ot : ot + 1], None, ALU.add
                    )

                for ot in range(ND):
                    for sh in range(2):
                        hTx = hT_own if sh == 0 else hT_oth
                        ps = psB.tile([P, 512], F32, tag="ps_k")
                        for kt in range(ND):
                            nc.tensor.matmul(
                                ps[:],
                                Wk_bf[:, kt, ot * P : (ot + 1) * P],
                                hTx[:, kt, :],
                                start=(kt == 0),
                                stop=(kt == ND - 1),
                            )
                        nc.scalar.activation(
                            out=KTe[0:64, ot, sh * 512 : (sh + 1) * 512],
                            in_=ps[0:64, :],
                            func=AF.Identity,
                            bias=bk_t[0:64, ot : ot + 1],
                        )
                        nc.vector.tensor_scalar(
                            KTo[64:128, ot, sh * 512 : (sh + 1) * 512],
                            ps[64:128, :],
                            bk_t[64:128, ot : ot + 1],
                            None,
                            ALU.add,
                        )
                for st in range(NS):
                    hTx = hT_own if st < 4 else hT_oth
                    st4 = st % 4
                    for oh in range(2):
                        ps = psB.tile([P, 512], F32, tag="ps_v")
                        for kt in range(ND):
                            nc.tensor.matmul(
                                ps[:],
                                hTx[:, kt, st4 * P : (st4 + 1) * P],
                                Wv_bf[:, kt, oh * 512 : (oh + 1) * 512],
                                start=(kt == 0),
                                stop=(kt == ND - 1),
                            )
                        nc.vector.tensor_tensor(
                            VN[:, st, oh * 8 : (oh + 1) * 8, 0:DH],
                            ps[:].rearrange("p (h e) -> p h e", h=8),
                            bv_bc[:, oh * 512 : (oh + 1) * 512].rearrange(
                                "p (h e) -> p h e", h=8
                            ),
                            ALU.add,
                        )

            # ------------------------------------------------- attention
            with ExitStack() as phC:
                wo_pool = phC.enter_context(tc.tile_pool(name="wo", bufs=1))
                stgC = phC.enter_context(tc.tile_pool(name="stgC", bufs=3))
                otp = phC.enter_context(tc.tile_pool(name="otp", bufs=1))

                bo_bc = _bcast_tile(nc, wo_pool, bo_e, D)
                Wo_bf = wo_pool.tile([P, ND, D], BF16, name="Wo_bf")
                for kt in range(ND):
                    s = stgC.tile([P, D], F32, tag="wstgC")
                    nc.sync.dma_start(out=s[:], in_=Wo_e[kt * P : (kt + 1) * P, :])
                    copy_cast(Wo_bf[:, kt, :], s[:], eng="g")

                # pre-bias the residual with bo (x + bo), in place
                for st in range(NSO):
                    nc.vector.tensor_tensor(
                        xN_own[:, st, :], xN_own[:, st, :], bo_bc[:], ALU.add
                    )

                OT = otp.tile([P, ND, SO], BF16, name="OT")

                phC1 = phC.enter_context(ExitStack())
                attn = phC1.enter_context(tc.tile_pool(name="attn", bufs=2))
                ps_s = phC1.enter_context(tc.tile_pool(name="ps_s", bufs=3, space="PSUM"))
                ps_o = phC1.enter_context(tc.tile_pool(name="ps_o", bufs=2, space="PSUM"))
                ps_bd = phC1.enter_context(tc.tile_pool(name="ps_bd", bufs=1, space="PSUM"))

                def normalize_pair(j, po_a, po_b):
                    # denominators live in psum row DH; broadcast 1/sum over
                    # all partitions with a zero-padded K=128 matmul vs e0
                    for off, po in ((0, po_a), (64, po_b)):
                        rec = attn.tile([P, SO], F32, tag="rec")
                        nc.gpsimd.memset(rec[:], 0.0)
                        lnrow = attn.tile([1, SO], F32, tag="lnrow")
                        nc.scalar.activation(
                            out=lnrow[:], in_=po[DH : DH + 1, :], func=AF.Ln
                        )
                        nc.scalar.activation(
                            out=rec[0:1, :], in_=lnrow[:], func=AF.Exp, scale=-1.0
                        )
                        psb2 = ps_bd.tile([P, SO], F32, tag="ps_b")
                        nc.tensor.matmul(psb2[:], e0[:], rec[:], start=True, stop=True)
                        bcast = attn.tile([64, SO], F32, tag="bcast")
                        nc.vector.tensor_copy(out=bcast[:], in_=psb2[0:64, :])
                        nc.vector.tensor_tensor(
                            OT[off : off + 64, j, :], po[0:DH, :], bcast[:], ALU.mult
                        )

                pending = None
                for j in range(H // 2):
                    pa = attn.tile([P, NS, SO], BF16, tag="probs_a")
                    pb = attn.tile([P, NS, SO], BF16, tag="probs_b")
                    po_a = ps_o.tile([P, SO], F32, tag="ps_oa")
                    po_b = ps_o.tile([P, SO], F32, tag="ps_ob")
                    for kb in range(NS):
                        psa = ps_s.tile([P, SO], F32, tag="ps_s")
                        psb = ps_s.tile([P, SO], F32, tag="ps_s")
                        nc.tensor.matmul(
                            psa[:],
                            KTe[:, j, kb * P : (kb + 1) * P],
                            QT[:, j, :],
                            start=True,
                            stop=True,
                        )
                        nc.tensor.matmul(
                            psb[:],
                            KTo[:, j, kb * P : (kb + 1) * P],
                            QT[:, j, :],
                            start=True,
                            stop=True,
                        )
                        nc.scalar.activation(
                            out=pa[:, kb, :], in_=psa[:], func=AF.Exp, scale=0.125
                        )
                        nc.scalar.activation(
                            out=pb[:, kb, :], in_=psb[:], func=AF.Exp, scale=0.125
                        )
                        nc.tensor.matmul(
                            po_a[:],
                            VN[:, kb, 2 * j, :],
                            pa[:, kb, :],
                            start=(kb == 0),
                            stop=(kb == NS - 1),
                        )
                        nc.tensor.matmul(
                            po_b[:],
                            VN[:, kb, 2 * j + 1, :],
                            pb[:, kb, :],
                            start=(kb == 0),
                            stop=(kb == NS - 1),
                        )
                    if pending is not None:
                        normalize_pair(*pending)
                    pending = (j, po_a, po_b)
                normalize_pair(*pending)
                phC1.close()

                # Wo projection, NATURAL output, fused residual:
                # x1[q, d] = (x + bo)[q, d] + sum_kt OT[:,kt,q].T @ Wo[kt, d]
                psD = phC.enter_context(tc.tile_pool(name="psD", bufs=2, space="PSUM"))
                for qb in range(NSO):
                    for dh in range(2):
                        ps = psD.tile([P, 512], F32, tag="ps_d")
                        for kt in range(ND):
                            nc.tensor.matmul(
                                ps[:],
                                OT[:, kt, qb * P : (qb + 1) * P],
                                Wo_bf[:, kt, dh * 512 : (dh + 1) * 512],
                                start=(kt == 0),
                                stop=(kt == ND - 1),
                            )
                        nc.vector.tensor_tensor(
                            x1N[:, qb, dh * 512 : (dh + 1) * 512],
                            xN_own[:, qb, dh * 512 : (dh + 1) * 512],
                            ps[:],
                            ALU.add,
                        )

            qkv_cm.__exit__(None, None, None)
            xown_cm.__exit__(None, None, None)

            # ----------------------------------------------- LN2 + MLP
            with ExitStack() as phF:
                h2p = phF.enter_context(tc.tile_pool(name="h2p", bufs=1))
                gtp = phF.enter_context(tc.tile_pool(name="gtp", bufs=1))
                wpp = phF.enter_context(tc.tile_pool(name="wpp", bufs=1))
                stgF = phF.enter_context(tc.tile_pool(name="stgF", bufs=4))
                wcst = phF.enter_context(tc.tile_pool(name="wcst", bufs=3))
                psF = phF.enter_context(tc.tile_pool(name="psF", bufs=2, space="PSUM"))
                opool = phF.enter_context(tc.tile_pool(name="opool", bufs=3))

                bp_bc = _bcast_tile(nc, h2p, bp_e, D)
                h2nN = h2p.tile([P, NSO, D], BF16, name="h2nN")
                h2T = h2p.tile([P, ND, SO], BF16, name="h2T")
                with tc.tile_pool(name="ln2", bufs=3) as lnp:
                    for st in range(NSO):
                        ln_tile(lnp, x1N[:, st, :], h2nN[:, st, :], eps_t, "l2")
                        for dt in range(ND):
                            pst = psF.tile([P, P], BF16, tag="ps_t2")
                            nc.tensor.transpose(
                                pst[:], h2nN[:, st, dt * P : (dt + 1) * P], ident[:]
                            )
                            nc.vector.tensor_scalar(
                                h2T[:, dt, st * P : (st + 1) * P],
                                pst[:],
                                ln2w_t[:, dt : dt + 1],
                                ln2b_t[:, dt : dt + 1],
                                ALU.mult,
                                ALU.add,
                            )

                GT = gtp.tile([P, NF, SO], BF16, name="GT")
                Wp_bf = wpp.tile([P, NF, D], BF16, name="Wp_bf")

                for ft in range(NF):
                    # stream + cast Wfc column block (split DMAs for queue ||)
                    sfc = stgF.tile([P, ND, P], F32, tag="sfc")
                    for hh in range(2):
                        nc.sync.dma_start(
                            out=sfc[:, hh * 4 : (hh + 1) * 4, :],
                            in_=Wfc_e[
                                hh * 512 : (hh + 1) * 512, ft * P : (ft + 1) * P
                            ].rearrange("(kt p) f -> p kt f", p=P),
                        )
                    wfc_bf = wcst.tile([P, ND, P], BF16, tag="wfc_bf")
                    copy_cast(wfc_bf[:], sfc[:])
                    # stream + cast Wproj row block
                    sp = stgF.tile([P, D], F32, tag="sp")
                    for hh in range(2):
                        nc.sync.dma_start(
                            out=sp[:, hh * 512 : (hh + 1) * 512],
                            in_=Wp_e[
                                ft * P : (ft + 1) * P, hh * 512 : (hh + 1) * 512
                            ],
                        )
                    copy_cast(Wp_bf[:, ft, :], sp[:])

                    ps = psF.tile([P, SO], F32, tag="ps_g")
                    for kt in range(ND):
                        nc.tensor.matmul(
                            ps[:],
                            wfc_bf[:, kt, :],
                            h2T[:, kt, :],
                            start=(kt == 0),
                            stop=(kt == ND - 1),
                        )
                    nc.scalar.activation(
                        out=GT[:, ft, :],
                        in_=ps[:],
                        func=AF.Gelu,
                        bias=bfc_t[:, ft : ft + 1],
                    )

                # pre-bias the residual with bproj (x1 + bproj), in place
                for st in range(NSO):
                    nc.vector.tensor_tensor(
                        x1N[:, st, :], x1N[:, st, :], bp_bc[:], ALU.add
                    )

                # proj, NATURAL output, fused residual:
                # out[s, d] = (x1 + bproj)[s, d] + sum_ft GT[:,ft,s].T @ Wp[ft, d]
                for qb in range(NSO):
                    for dh in range(2):
                        ps = psF.tile([P, 512], F32, tag="ps_p")
                        for ft in range(NF):
                            nc.tensor.matmul(
                                ps[:],
                                GT[:, ft, qb * P : (qb + 1) * P],
                                Wp_bf[:, ft, dh * 512 : (dh + 1) * 512],
                                start=(ft == 0),
                                stop=(ft == NF - 1),
                            )
                        of = opool.tile([P, 512], F32, tag="of")
                        nc.vector.tensor_tensor(
                            of[:],
                            x1N[:, qb, dh * 512 : (dh + 1) * 512],
                            ps[:],
                            ALU.add,
                        )
                        nc.sync.dma_start(
                            out=out_ext[qb * P : (qb + 1) * P, dh * 512 : (dh + 1) * 512],
                            in_=of[:],
                        )

    _split_multiwaits(nc)
    return nc


_NC_CACHE = None


def _get_nc():
    global _NC_CACHE
    if _NC_CACHE is None:
        _NC_CACHE = build()
    return _NC_CACHE


def make_in_maps(inputs):
    """Shard FULL inputs into per-core input maps (own rows rotated first)."""
    x = np.asarray(inputs["x"], dtype=np.float32)
    names = [
        "ln1_w", "ln1_b", "Wq", "bq", "Wk", "bk", "Wv", "bv", "Wo", "bo",
        "ln2_w", "ln2_b", "Wfc", "bfc", "Wproj", "bproj",
    ]
    shared = {n: np.ascontiguousarray(np.asarray(inputs[n], dtype=np.float32))
              for n in names}
    in_maps = []
    for c in range(N_CORES):
        b, half = c // 2, c % 2
        xb = x[b]
        x_core = np.concatenate(
            [xb[half * SO : (half + 1) * SO], xb[(1 - half) * SO : (2 - half) * SO]],
            axis=0,
        )
        m = {"x": np.ascontiguousarray(x_core)}
        m.update(shared)
        in_maps.append(m)
    return in_maps


def kernel(**inputs) -> np.ndarray:
    from concourse.bass_utils import run_bass_kernel_spmd

    nc = _get_nc()
    in_maps = make_in_maps(inputs)
    res = run_bass_kernel_spmd(nc, in_maps, list(range(N_CORES)))
    B = 4
    out = np.empty((B, S, D), dtype=np.float32)
    for c in range(N_CORES):
        b, half = c // 2, c % 2
        out[b, half * SO : (half + 1) * SO] = res.results[c]["out"]
    return out



# revision 15
# speedup vs baseline: 1.0197x; 1.0197x over previous
"""Trainium2 Bass kernel for a dense transformer block (B=4, N=1024, D=1024,
H=16, Dh=64, MLP 4x), distributed over 8 NeuronCores with ZERO collectives.

Sharding: core c handles batch b = c//2, sequence half = c%2 (512 query
rows).  K/V are computed for the batch's full 1024-token sequence on both
cores of a pair; the sequence is rotated per-core so the core's own 512 rows
are always rows 0..511 of its input, so all 8 cores run one identical SPMD
program.

v2 layout/scheduling notes:
- All weights are cast to bf16 AND pre-packed into their exact SBUF layouts
  on the HOST, so every weight DMA is a contiguous [128, N] blast and no
  on-chip casting is needed (v1 spent ~200us of engine time on casts and
  streamed 48MB of f32 weights through one DMA queue).
- x is passed twice: full-sequence bf16 (feeds LN1) and own-half f32 (the
  residual).  LN1 stats tolerate bf16 input easily at the 2e-2 gate.
- V (then Q) matmuls are interleaved into the LN1 loop so the PE is busy
  from ~5us on; K projection is pipelined per-head-pair against the
  attention inner loop so the ACT-bound softmax EXP (~0.7us per [128,512]
  tile, the real attention bottleneck) overlaps PE matmul work.
- Scores contract over Dh=64: the two heads of a d-tile run as two
  CONCURRENT K=64 matmuls in distinct row-groups of the PE array
  (tile_position (0,0)/(64,0)), halving score PE time.
- Softmax denominators ride in column 64 of V (ones column); the
  normalization reciprocal-broadcast runs on ACT+GPSIMD+DVE, not PE.
"""

import numpy as np

import bass_rust
import concourse.bass as bass
import concourse.mybir as mybir
import concourse.tile as tile
from concourse import library_config
from concourse.masks import make_identity

F32 = mybir.dt.float32
BF16 = mybir.dt.bfloat16
AF = mybir.ActivationFunctionType
ALU = mybir.AluOpType

P = 128
D = 1024
S = 1024          # full sequence (per batch)
SO = 512          # own rows per core
H = 16
DH = 64
F = 4096
EPS = 1e-5
N_CORES = 8

ND = D // P       # 8   d tiles
NS = S // P       # 8   full-seq tiles
NSO = SO // P     # 4   own-seq tiles
NF = F // P       # 32  ff tiles


# --------------------------------------------------------------------------
# Workaround: this compiler build supports only ONE semaphore wait per
# instruction.  Move excess waits onto fresh NOPs inserted just before the
# offending instruction on the same engine.
# --------------------------------------------------------------------------
_counter = [0]


def _split_multiwaits(nc):
    nsplit = 0
    for fn in nc.m.functions:
        for blk in fn.blocks:
            il = list(blk.instructions)
            out = []
            changed = False
            for inst in il:
                si = inst.sync_info
                if si is not None and len(si.on_wait) > 1:
                    waits = list(si.on_wait)
                    for w in waits[:-1]:
                        _counter[0] += 1
                        nop = mybir.InstNoOp(
                            name=f"I-waitsplit-{_counter[0]}", ins=[], outs=[]
                        )
                        nop.engine = inst.engine
                        nop.sync_info = bass_rust.SyncInfo(on_wait=[w], on_update=[])
                        out.append(nop)
                        nc.register_instruction(nop, overwrite=True)
                    inst.sync_info = bass_rust.SyncInfo(
                        on_wait=[waits[-1]], on_update=list(si.on_update)
                    )
                    changed = True
                    nsplit += 1
                out.append(inst)
            if changed:
                blk.instructions = out
    return nsplit


def _vec_tile(nc, pool, ext, n):
    """Load a [n*128] dram vector as a [128, n] sbuf tile (col i = tile i)."""
    t = pool.tile([P, n], F32, name=ext.name + "_sb")
    nc.scalar.dma_start(out=t[:], in_=ext[:].rearrange("(o p) -> p o", p=P))
    return t


def _bcast_tile(nc, pool, ext, n):
    """Load a [n] dram vector broadcast to a [128, n] sbuf tile."""
    t = pool.tile([P, n], F32, name=ext.name + "_bc")
    ap = ext[:]
    src = bass.AP(tensor=ap.tensor, offset=ap.offset, ap=[[0, P], ap.ap[0]])
    nc.scalar.dma_start(out=t[:], in_=src)
    return t


def build():
    nc = bass.Bass(name="tfblock")

    xbf_ext = nc.declare_dram_parameter("x_bf", [S, D], BF16, isOutput=False)
    xo_ext = nc.declare_dram_parameter("x_own", [SO, D], F32, isOutput=False)
    ln1_w = nc.declare_dram_parameter("ln1_w", [D], F32, isOutput=False)
    ln1_b = nc.declare_dram_parameter("ln1_b", [D], F32, isOutput=False)
    wq_ext = nc.declare_dram_parameter("wq_p", [P, ND * D], BF16, isOutput=False)
    bq_e = nc.declare_dram_parameter("bq", [D], F32, isOutput=False)
    wk_ext = nc.declare_dram_parameter("wk_p", [P, ND * D], BF16, isOutput=False)
    bk_e = nc.declare_dram_parameter("bk", [D], F32, isOutput=False)
    wv_ext = nc.declare_dram_parameter("wv_p", [P, 2 * ND * SO], BF16, isOutput=False)
    bv_e = nc.declare_dram_parameter("bv", [D], F32, isOutput=False)
    wo_ext = nc.declare_dram_parameter("wo_p", [P, ND * D], BF16, isOutput=False)
    bo_e = nc.declare_dram_parameter("bo", [D], F32, isOutput=False)
    ln2_w = nc.declare_dram_parameter("ln2_w", [D], F32, isOutput=False)
    ln2_b = nc.declare_dram_parameter("ln2_b", [D], F32, isOutput=False)
    wfc_ext = nc.declare_dram_parameter("wfc_p", [P, NF * ND * P], BF16, isOutput=False)
    bfc_e = nc.declare_dram_parameter("bfc", [F], F32, isOutput=False)
    wp_ext = nc.declare_dram_parameter("wp_p", [P, NF * D], BF16, isOutput=False)
    bp_e = nc.declare_dram_parameter("bproj", [D], F32, isOutput=False)
    out_ext = nc.declare_dram_parameter("out", [SO, D], F32, isOutput=True)

    def ln_tile(lnp, src_ap, hn_out, eps_t, tag):
        """LayerNorm stats on DVE + apply on ACT: hn_out = (src-mu)*rstd."""
        stats = lnp.tile([P, 2, 6], F32, tag=tag + "_st")
        for g in range(2):
            nc.vector.bn_stats(out=stats[:, g, :], in_=src_ap[:, g * 512 : (g + 1) * 512])
        mv = lnp.tile([P, 2], F32, tag=tag + "_mv")
        nc.vector.bn_aggr(out=mv[:], in_=stats[:])
        lnv = lnp.tile([P, 1], F32, tag=tag + "_sd")
        nc.scalar.activation(out=lnv[:], in_=mv[:, 1:2], func=AF.Ln, bias=eps_t[:])
        rstd = lnp.tile([P, 1], F32, tag=tag + "_rs")
        nc.scalar.activation(out=rstd[:], in_=lnv[:], func=AF.Exp, scale=-0.5)
        nb = lnp.tile([P, 1], F32, tag=tag + "_nb")
        nc.vector.tensor_scalar(nb[:], mv[:, 0:1], rstd[:], -1.0, ALU.mult, ALU.mult)
        nc.scalar.activation(
            out=hn_out, in_=src_ap, func=AF.Identity, bias=nb[:], scale=rstd[:]
        )

    with tile.TileContext(nc) as tc:
        from contextlib import ExitStack

        with ExitStack() as top:
            consts = top.enter_context(tc.tile_pool(name="consts", bufs=1))
            persist = top.enter_context(tc.tile_pool(name="persist", bufs=1))

            # consts travel on the ACT hwdge queue; weights/x on the sync queue
            ln1w_t = _vec_tile(nc, consts, ln1_w, ND)
            ln1b_t = _vec_tile(nc, consts, ln1_b, ND)
            ln2w_t = _vec_tile(nc, consts, ln2_w, ND)
            ln2b_t = _vec_tile(nc, consts, ln2_b, ND)
            bq_t = _vec_tile(nc, consts, bq_e, ND)
            bk_t = _vec_tile(nc, consts, bk_e, ND)
            bfc_t = _vec_tile(nc, consts, bfc_e, NF)
            bv_bc = _bcast_tile(nc, consts, bv_e, D)
            bo_bc = _bcast_tile(nc, consts, bo_e, D)
            bp_bc = _bcast_tile(nc, consts, bp_e, D)

            eps_t = consts.tile([P, 1], F32, name="eps")
            nc.vector.memset(eps_t[:], EPS)
            ident = consts.tile([P, P], BF16, name="ident")
            make_identity(nc, ident[:])
            e0 = consts.tile([P, P], F32, name="e0")
            nc.vector.memset(e0[:], 0.0)
            nc.vector.memset(e0[0:1, :], 1.0)

            # residual stream (own half), f32, on the ACT queue (off the
            # critical startup path of the sync queue)
            xN_own = persist.tile([P, NSO, D], F32, name="xN_own")
            nc.scalar.dma_start(
                out=xN_own[:], in_=xo_ext[:].rearrange("(t p) d -> p t d", p=P)
            )

            # long-lived mid tensors: Wo (consumed in the Wo projection),
            # OT (attention out, transposed), h2T (LN2 out, transposed)
            mid_cm = tc.tile_pool(name="mid", bufs=1)
            midp = mid_cm.__enter__()
            Wo_sb = midp.tile([P, ND, D], BF16, name="Wo_sb")
            OT = midp.tile([P, ND, SO], BF16, name="OT")
            h2T = midp.tile([P, ND, SO], BF16, name="h2T")

            # pre-bias the residual with bo (x + bo), in place
            for st in range(NSO):
                nc.vector.tensor_tensor(
                    xN_own[:, st, :], xN_own[:, st, :], bo_bc[:], ALU.add
                )

            # transposed-LN1 outputs + QKV live until end of attention
            hT_cm = tc.tile_pool(name="hTp", bufs=1)
            hTp = hT_cm.__enter__()
            hT_own = hTp.tile([P, ND, SO], BF16, name="hT_own")
            hT_oth = hTp.tile([P, ND, SO], BF16, name="hT_oth")

            qkv_cm = tc.tile_pool(name="qkvp", bufs=1)
            qkvp = qkv_cm.__enter__()
            QT = qkvp.tile([P, ND, SO], BF16, name="QT")
            KT = qkvp.tile([P, ND, S], BF16, name="KT")
            VN = qkvp.tile([P, NS, H, 66], BF16, name="VN")
            nc.vector.memset(VN[:, :, :, DH : DH + 1], 1.0)

            wk_cm = tc.tile_pool(name="wkp", bufs=1)
            wkp = wk_cm.__enter__()
            Wk_sb = wkp.tile([P, ND, D], BF16, name="Wk_sb")

            # ------------------------- phase B: x + W DMAs, LN1, V, Q
            with ExitStack() as phB:
                wv_pool = phB.enter_context(tc.tile_pool(name="wvp", bufs=1))
                xbf_pool = phB.enter_context(tc.tile_pool(name="xbfp", bufs=1))
                hnp = phB.enter_context(tc.tile_pool(name="hnp", bufs=3))
                lnp = phB.enter_context(tc.tile_pool(name="ln1", bufs=2))
                ps_t = phB.enter_context(tc.tile_pool(name="ps_t", bufs=2, space="PSUM"))
                ps_v = phB.enter_context(tc.tile_pool(name="ps_v", bufs=2, space="PSUM"))
                ps_q = phB.enter_context(tc.tile_pool(name="ps_q", bufs=2, space="PSUM"))

                Wv_sb = wv_pool.tile([P, 2, ND, SO], BF16, name="Wv_sb")
                Wq_sb = wv_pool.tile([P, ND, D], BF16, name="Wq_sb")
                xbf = xbf_pool.tile([P, NS, D], BF16, name="xbf")

                # sync-queue DMA order is the startup schedule: x tiles for
                # the own half, first V weight half, rest of x, V second
                # half, then Wq/Wk/Wo.
                for st in range(4):
                    nc.sync.dma_start(
                        out=xbf[:, st, :], in_=xbf_ext[st * P : (st + 1) * P, :]
                    )
                nc.sync.dma_start(
                    out=Wv_sb[:, 0, :, :],
                    in_=wv_ext[:, 0 : ND * SO].rearrange("p (k c) -> p k c", k=ND),
                )
                for st in range(4, NS):
                    nc.sync.dma_start(
                        out=xbf[:, st, :], in_=xbf_ext[st * P : (st + 1) * P, :]
                    )
                nc.sync.dma_start(
                    out=Wv_sb[:, 1, :, :],
                    in_=wv_ext[:, ND * SO :].rearrange("p (k c) -> p k c", k=ND),
                )
                nc.sync.dma_start(
                    out=Wq_sb[:], in_=wq_ext[:].rearrange("p (k c) -> p k c", k=ND)
                )
                nc.sync.dma_start(
                    out=Wk_sb[:], in_=wk_ext[:].rearrange("p (k c) -> p k c", k=ND)
                )
                nc.sync.dma_start(
                    out=Wo_sb[:], in_=wo_ext[:].rearrange("p (k c) -> p k c", k=ND)
                )

                def emit_v(st):
                    hTx = hT_own if st < 4 else hT_oth
                    st4 = st % 4
                    for oh in range(2):
                        ps = ps_v.tile([P, SO], F32, tag="ps_v")
                        for kt in range(ND):
                            nc.tensor.matmul(
                                ps[:],
                                hTx[:, kt, st4 * P : (st4 + 1) * P],
                                Wv_sb[:, oh, kt, :],
                                start=(kt == 0),
                                stop=(kt == ND - 1),
                            )
                        nc.vector.tensor_tensor(
                            VN[:, st, oh * 8 : (oh + 1) * 8, 0:DH],
                            ps[:].rearrange("p (h e) -> p h e", h=8),
                            bv_bc[:, oh * 512 : (oh + 1) * 512].rearrange(
                                "p (h e) -> p h e", h=8
                            ),
                            ALU.add,
                        )

                # LN1 per tile; h^T PE-transposes (ln1 w/b fused in the DVE
                # copy-back) run one tile ahead of the V matmuls
                for st in range(NS):
                    hn = hnp.tile([P, D], BF16, tag="hn")
                    ln_tile(lnp, xbf[:, st, :], hn[:], eps_t, "l1")
                    hTx = hT_own if st < 4 else hT_oth
                    st4 = st % 4
                    pst = ps_t.tile([P, ND, P], BF16, tag="ps_t")
                    for dt in range(ND):
                        nc.tensor.transpose(
                            pst[:, dt, :], hn[:, dt * P : (dt + 1) * P], ident[:]
                        )
                        nc.vector.tensor_scalar(
                            hTx[:, dt, st4 * P : (st4 + 1) * P],
                            pst[:, dt, :],
                            ln1w_t[:, dt : dt + 1],
                            ln1b_t[:, dt : dt + 1],
                            ALU.mult,
                            ALU.add,
                        )
                    if st >= 1:
                        emit_v(st - 1)
                emit_v(NS - 1)

                for ot in range(ND):
                    ps = ps_q.tile([P, SO], F32, tag="ps_q")
                    for kt in range(ND):
                        nc.tensor.matmul(
                            ps[:],
                            Wq_sb[:, kt, ot * P : (ot + 1) * P],
                            hT_own[:, kt, :],
                            start=(kt == 0),
                            stop=(kt == ND - 1),
                        )
                    nc.vector.tensor_scalar(
                        QT[:, ot, :], ps[:], bq_t[:, ot : ot + 1], None, ALU.add
                    )

            # ------------------- pipeline: K_j projection + attention_j
            with ExitStack() as phC:
                attn = phC.enter_context(tc.tile_pool(name="attn", bufs=1))
                prp = phC.enter_context(tc.tile_pool(name="prp", bufs=3))
                ps_k = phC.enter_context(tc.tile_pool(name="ps_k", bufs=1, space="PSUM"))
                ps_s = phC.enter_context(tc.tile_pool(name="ps_s", bufs=2, space="PSUM"))
                ps_o = phC.enter_context(tc.tile_pool(name="ps_o", bufs=1, space="PSUM"))

                def emit_k(j):
                    psk = ps_k.tile([P, 2, SO], F32, tag="ps_k")
                    for sh in range(2):
                        hTx = hT_own if sh == 0 else hT_oth
                        for kt in range(ND):
                            nc.tensor.matmul(
                                psk[:, sh, :],
                                Wk_sb[:, kt, j * P : (j + 1) * P],
                                hTx[:, kt, :],
                                start=(kt == 0),
                                stop=(kt == ND - 1),
                            )
                        nc.vector.tensor_scalar(
                            KT[:, j, sh * SO : (sh + 1) * SO],
                            psk[:, sh, :],
                            bk_t[:, j : j + 1],
                            None,
                            ALU.add,
                        )

                rec = attn.tile([P, 2, SO], F32, name="rec")
                nc.gpsimd.memset(rec[:], 0.0)

                emit_k(0)
                for j in range(ND):
                    if j + 1 < ND:
                        emit_k(j + 1)
                    po = ps_o.tile([P, 2, SO], F32, tag="ps_o")
                    for kb in range(NS):
                        pss = ps_s.tile([P, 2, SO], F32, tag="ps_s")
                        nc.tensor.matmul(
                            pss[:, 0, :],
                            KT[0:DH, j, kb * P : (kb + 1) * P],
                            QT[0:DH, j, :],
                            start=True,
                            stop=True,
                        )
                        nc.tensor.matmul(
                            pss[:, 1, :],
                            KT[DH:P, j, kb * P : (kb + 1) * P],
                            QT[DH:P, j, :],
                            start=True,
                            stop=True,
                        )
                        prob = prp.tile([P, 2, SO], BF16, tag="prob")
                        nc.scalar.activation(
                            out=prob[:], in_=pss[:], func=AF.Exp, scale=0.125
                        )
                        nc.tensor.matmul(
                            po[0:65, 0, :],
                            VN[:, kb, 2 * j, 0:65],
                            prob[:, 0, :],
                            start=(kb == 0),
                            stop=(kb == NS - 1),
                        )
                        nc.tensor.matmul(
                            po[0:65, 1, :],
                            VN[:, kb, 2 * j + 1, 0:65],
                            prob[:, 1, :],
                            start=(kb == 0),
                            stop=(kb == NS - 1),
                        )
                    # normalize: denominators live in psum row DH; 1/Z on
                    # DVE, broadcast over partitions with a zero-padded K=128
                    # matmul vs e0 (into the shared ps_s pool), multiply on
                    # DVE with partition-shifted writes into OT halves.
                    nc.vector.reciprocal(rec[0:1, :, :], po[DH : DH + 1, :, :])
                    psb = ps_s.tile([P, 2, SO], F32, tag="ps_s")
                    for hh in range(2):
                        nc.tensor.matmul(
                            psb[:, hh, :], e0[:], rec[:, hh, :],
                            start=True, stop=True,
                        )
                    bcast = attn.tile([DH, 2, SO], F32, tag="bcast")
                    nc.vector.tensor_copy(out=bcast[:], in_=psb[0:DH, :, :])
                    nc.vector.tensor_tensor(
                        OT[0:DH, j, :], po[0:DH, 0, :], bcast[:, 0, :], ALU.mult
                    )
                    nc.vector.tensor_tensor(
                        OT[DH:P, j, :], po[0:DH, 1, :], bcast[:, 1, :], ALU.mult
                    )

            wk_cm.__exit__(None, None, None)
            qkv_cm.__exit__(None, None, None)
            hT_cm.__exit__(None, None, None)

            # ------------------- Wo projection + LN2 + h^T, fused residual
            x1_cm = tc.tile_pool(name="x1p", bufs=1)
            x1p = x1_cm.__enter__()
            x1N = x1p.tile([P, NSO, D], F32, name="x1N")

            with ExitStack() as phD:
                lnp2 = phD.enter_context(tc.tile_pool(name="ln2", bufs=2))
                hn2p = phD.enter_context(tc.tile_pool(name="hn2p", bufs=2))
                psD = phD.enter_context(tc.tile_pool(name="psD", bufs=2, space="PSUM"))
                ps_t2 = phD.enter_context(
                    tc.tile_pool(name="ps_t2", bufs=2, space="PSUM")
                )

                hn2_tiles = {}

                def emit_t2(qb):
                    hn2 = hn2_tiles.pop(qb)
                    pst2 = ps_t2.tile([P, ND, P], BF16, tag="ps_t2")
                    for dt in range(ND):
                        nc.tensor.transpose(
                            pst2[:, dt, :], hn2[:, dt * P : (dt + 1) * P], ident[:]
                        )
                        nc.vector.tensor_scalar(
                            h2T[:, dt, qb * P : (qb + 1) * P],
                            pst2[:, dt, :],
                            ln2w_t[:, dt : dt + 1],
                            ln2b_t[:, dt : dt + 1],
                            ALU.mult,
                            ALU.add,
                        )

                for qb in range(NSO):
                    for dh in range(2):
                        ps = psD.tile([P, 512], F32, tag="ps_d")
                        for kt in range(ND):
                            nc.tensor.matmul(
                                ps[:],
                                OT[:, kt, qb * P : (qb + 1) * P],
                                Wo_sb[:, kt, dh * 512 : (dh + 1) * 512],
                                start=(kt == 0),
                                stop=(kt == ND - 1),
                            )
                        nc.vector.tensor_tensor(
                            x1N[:, qb, dh * 512 : (dh + 1) * 512],
                            xN_own[:, qb, dh * 512 : (dh + 1) * 512],
                            ps[:],
                            ALU.add,
                        )
                    # LN2 for this row-block, then pre-bias x1N with bproj;
                    # h^T transposes trail one block so the PE never waits
                    # on the LN2 DVE/ACT chain.
                    hn2 = hn2p.tile([P, D], BF16, tag="hn2")
                    ln_tile(lnp2, x1N[:, qb, :], hn2[:], eps_t, "l2")
                    nc.vector.tensor_tensor(
                        x1N[:, qb, :], x1N[:, qb, :], bp_bc[:], ALU.add
                    )
                    hn2_tiles[qb] = hn2
                    if qb >= 1:
                        emit_t2(qb - 1)
                emit_t2(NSO - 1)

            # ----------------------------------------------- MLP
            with ExitStack() as phF:
                gtp = phF.enter_context(tc.tile_pool(name="gtp", bufs=1))
                wpp = phF.enter_context(tc.tile_pool(name="wpp", bufs=1))
                wcst = phF.enter_context(tc.tile_pool(name="wcst", bufs=4))
                psF = phF.enter_context(tc.tile_pool(name="psF", bufs=2, space="PSUM"))
                psP = phF.enter_context(tc.tile_pool(name="psP", bufs=2, space="PSUM"))
                opool = phF.enter_context(tc.tile_pool(name="opool", bufs=3))

                GT = gtp.tile([P, NF, SO], BF16, name="GT")
                Wp_sb = wpp.tile([P, NF, D], BF16, name="Wp_sb")

                for ft in range(NF):
                    wfc = wcst.tile([P, ND, P], BF16, tag="wfc")
                    nc.sync.dma_start(
                        out=wfc[:],
                        in_=wfc_ext[:, ft * D : (ft + 1) * D].rearrange(
                            "p (k f) -> p k f", k=ND
                        ),
                    )
                    nc.sync.dma_start(
                        out=Wp_sb[:, ft, :], in_=wp_ext[:, ft * D : (ft + 1) * D]
                    )
                    ps = psF.tile([P, SO], F32, tag="ps_g")
                    for kt in range(ND):
                        nc.tensor.matmul(
                            ps[:],
                            wfc[:, kt, :],
                            h2T[:, kt, :],
                            start=(kt == 0),
                            stop=(kt == ND - 1),
                        )
                    nc.scalar.activation(
                        out=GT[:, ft, :],
                        in_=ps[:],
                        func=AF.Gelu,
                        bias=bfc_t[:, ft : ft + 1],
                    )

                # proj, NATURAL output, fused residual:
                # out[s, d] = (x1 + bproj)[s, d] + sum_ft GT[:,ft,s].T @ Wp[ft, d]
                for qb in range(NSO):
                    for dh in range(2):
                        ps = psP.tile([P, 512], F32, tag="ps_p")
                        for ft in range(NF):
                            nc.tensor.matmul(
                                ps[:],
                                GT[:, ft, qb * P : (qb + 1) * P],
                                Wp_sb[:, ft, dh * 512 : (dh + 1) * 512],
                                start=(ft == 0),
                                stop=(ft == NF - 1),
                            )
                        of = opool.tile([P, 512], F32, tag="of")
                        nc.vector.tensor_tensor(
                            of[:],
                            x1N[:, qb, dh * 512 : (dh + 1) * 512],
                            ps[:],
                            ALU.add,
                        )
                        nc.sync.dma_start(
                            out=out_ext[qb * P : (qb + 1) * P, dh * 512 : (dh + 1) * 512],
                            in_=of[:],
                        )

            x1_cm.__exit__(None, None, None)
            mid_cm.__exit__(None, None, None)

    _split_multiwaits(nc)
    return nc


_NC_CACHE = None


def _get_nc():
    global _NC_CACHE
    if _NC_CACHE is None:
        _NC_CACHE = build()
    return _NC_CACHE


def _pack_weights(inputs):
    """Host-side: cast weights to bf16 and pre-arrange into SBUF layouts."""
    import ml_dtypes

    bf = ml_dtypes.bfloat16
    Wq = np.asarray(inputs["Wq"], np.float32)
    Wk = np.asarray(inputs["Wk"], np.float32)
    Wv = np.asarray(inputs["Wv"], np.float32)
    Wo = np.asarray(inputs["Wo"], np.float32)
    Wfc = np.asarray(inputs["Wfc"], np.float32)
    Wp = np.asarray(inputs["Wproj"], np.float32)

    def pack_dd(W):  # [D, D] -> [P, ND*D], [p, kt*D+c] = W[kt*P+p, c]
        return np.ascontiguousarray(
            W.reshape(ND, P, D).transpose(1, 0, 2).reshape(P, ND * D).astype(bf)
        )

    wq_p = pack_dd(Wq)
    wk_p = pack_dd(Wk)
    wo_p = pack_dd(Wo)
    # [p, oh*ND*SO + kt*SO + c] = Wv[kt*P+p, oh*SO+c]
    wv_p = np.ascontiguousarray(
        Wv.reshape(ND, P, 2, SO).transpose(1, 2, 0, 3).reshape(P, 2 * ND * SO).astype(bf)
    )
    # [p, ft*ND*P + kt*P + f] = Wfc[kt*P+p, ft*P+f]
    wfc_p = np.ascontiguousarray(
        Wfc.reshape(ND, P, NF, P).transpose(1, 2, 0, 3).reshape(P, NF * ND * P).astype(bf)
    )
    # [p, ft*D + c] = Wp[ft*P+p, c]
    wp_p = np.ascontiguousarray(
        Wp.reshape(NF, P, D).transpose(1, 0, 2).reshape(P, NF * D).astype(bf)
    )
    return {
        "wq_p": wq_p, "wk_p": wk_p, "wv_p": wv_p, "wo_p": wo_p,
        "wfc_p": wfc_p, "wp_p": wp_p,
    }


def make_in_maps(inputs):
    """Shard FULL inputs into per-core input maps (own rows rotated first)."""
    import ml_dtypes

    bf = ml_dtypes.bfloat16
    x = np.asarray(inputs["x"], dtype=np.float32)
    vec_names = [
        "ln1_w", "ln1_b", "bq", "bk", "bv", "bo", "ln2_w", "ln2_b", "bfc", "bproj",
    ]
    shared = {n: np.ascontiguousarray(np.asarray(inputs[n], dtype=np.float32))
              for n in vec_names}
    shared.update(_pack_weights(inputs))
    in_maps = []
    for c in range(N_CORES):
        b, half = c // 2, c % 2
        xb = x[b]
        x_core = np.concatenate(
            [xb[half * SO : (half + 1) * SO], xb[(1 - half) * SO : (2 - half) * SO]],
            axis=0,
        )
        m = {
            "x_bf": np.ascontiguousarray(x_core.astype(bf)),
            "x_own": np.ascontiguousarray(x_core[0:SO]),
        }
        m.update(shared)
        in_maps.append(m)
    return in_maps


def kernel(**inputs) -> np.ndarray:
    from concourse.bass_utils import run_bass_kernel_spmd

    nc = _get_nc()
    in_maps = make_in_maps(inputs)
    res = run_bass_kernel_spmd(nc, in_maps, list(range(N_CORES)))
    B = 4
    out = np.empty((B, S, D), dtype=np.float32)
    for c in range(N_CORES):
        b, half = c // 2, c % 2
        out[b, half * SO : (half + 1) * SO] = res.results[c]["out"]
    return out
